# revision 12
# baseline (speedup 1.0000x reference)
"""Batched Householder reflection: s_new[b] = s[b] - 2*(v[b]@s[b])/(v[b]@v[b]) * v[b].

Full inputs v, s: [512, 512] f32. Sharded batch-parallel across 8 NeuronCores
(64 rows per core). Per core: rows on SBUF partitions, K=512 on the free axis.
v and s shards are stacked host-side into one [2, 64, 512] DRAM tensor.

v5 schedule (perfetto-driven, refines the v1 structure):
- 4 load streams: SP v[0:40], ACT s[0:40], pool v[40:64] THEN s[40:64]
  (pool's two dma_starts share one SWDGE ring and serialize; v goes first
  so ACT's Square starts earlier; SP/ACT carry more lines since their
  HWDGE desc-gen is parallel).
- ACT order: load desc-gen, then a warm Square on const zeros. The warm
  pulls the 1283ns ACT table load into the load window; the real square
  then starts as soon as v lands (sv>=32), accum_out -> nsq.
- DVE: dot (with -2 folded via op0 scalar), reciprocal, coef, final.
  3 semaphores: sv (v-loads + DVE chain), ss (s-loads + nsq visibility),
  so (store completions). Stores: SP 28 / ACT 12 / pool 24 (ACT's store
  desc-gen is the slowest, pool's starts latest).
- SP waits so>=48 (all stores landed, which also proves ACT/pool passed
  their sv-waits) then clears all three sems for NEFF re-execution.
"""

import numpy as np

B, K = 512, 512
N_CORES = 8
B_LOC = B // N_CORES  # 64 rows per core

_nc = None


def _build():
    import concourse.bass as bass
    from concourse import mybir

    nc = bass.Bass("TRN2", debug=False, num_devices=N_CORES, num_swdge_queues=1)
    f32 = mybir.dt.float32

    vs = nc.dram_tensor("vs", [2, B_LOC, K], f32, kind="ExternalInput").ap()
    out = nc.dram_tensor("out", [B_LOC, K], f32, kind="ExternalOutput").ap()

    vs_t = nc.alloc_sbuf_tensor("vs_t", [B_LOC, 2, K], f32).ap()
    o_t = nc.alloc_sbuf_tensor("o_t", [B_LOC, K], f32).ap()
    junk_vs = nc.alloc_sbuf_tensor("junk_vs", [B_LOC, K], f32).ap()
    junk_vv = nc.alloc_sbuf_tensor("junk_vv", [B_LOC, K], f32).ap()
    warm = nc.alloc_sbuf_tensor("warm", [B_LOC, 1], f32).ap()
    dotm2 = nc.alloc_sbuf_tensor("dotm2", [B_LOC, 1], f32).ap()
    nsq = nc.alloc_sbuf_tensor("nsq", [B_LOC, 1], f32).ap()
    coef = nc.alloc_sbuf_tensor("coef", [B_LOC, 1], f32).ap()
    rcp = nc.alloc_sbuf_tensor("rcp", [B_LOC, 1], f32).ap()

    sv = nc.alloc_semaphore("sv")
    ss = nc.alloc_semaphore("ss")
    so = nc.alloc_semaphore("so")

    mult = mybir.AluOpType.mult
    add = mybir.AluOpType.add
    Square = mybir.ActivationFunctionType.Square

    sp, act, ve, pl = nc.sync, nc.scalar, nc.vector, nc.gpsimd
    v_t = vs_t[:, 0, :]
    s_t = vs_t[:, 1, :]
    zero64 = nc.const_aps.scalar_like(0.0, dotm2[:])

    # ---- loads: v first everywhere. ACT capped at 32 lines (its HWDGE
    # desc-gen falls off a cliff above ~32: 683ns@32 vs 1682ns@40, which
    # also pushes the warm-triggered table load past v-arrival). The other
    # 32 s-rows ride as small second dma_starts on SP and pool. ----
    sp.dma_start(out=vs_t[:44, 0, :], in_=vs[0, :44, :]).then_inc(sv, 16)
    sp.dma_start(out=vs_t[32:52, 1, :], in_=vs[1, 32:52, :]).then_inc(ss, 16)
    act.dma_start(out=vs_t[:32, 1, :], in_=vs[1, :32, :]).then_inc(ss, 16)
    act.activation(out=warm[:], in_=zero64, func=Square)  # pulls table load early
    pl.dma_start(out=vs_t[44:, 0, :], in_=vs[0, 44:, :]).then_inc(sv, 16)
    pl.dma_start(out=vs_t[52:, 1, :], in_=vs[1, 52:, :]).then_inc(ss, 16)

    # nsq = rowsum(v*v) on ACT as soon as v lands
    act.wait_ge(sv, 32)
    act.activation(out=junk_vv[:], in_=v_t, func=Square, accum_out=nsq[:]).then_inc(
        ss, 1
    )

    # DVE chain; sv also tracks DVE write visibility
    ve.wait_ge(sv, 32)
    ve.wait_ge(ss, 48)
    ve.scalar_tensor_tensor(
        out=junk_vs[:],
        in0=v_t,
        scalar=-2.0,
        in1=s_t,
        op0=mult,
        op1=mult,
        accum_out=dotm2[:],
    ).then_inc(sv, 1)
    ve.wait_ge(ss, 49)
    ve.reciprocal(out=rcp[:], in_=nsq[:]).then_inc(sv, 1)
    ve.wait_ge(sv, 34)
    ve.scalar_tensor_tensor(
        out=coef[:], in0=dotm2[:], scalar=1.0, in1=rcp[:], op0=mult, op1=mult
    ).then_inc(sv, 1)
    ve.wait_ge(sv, 35)
    ve.scalar_tensor_tensor(
        out=o_t[:],
        in0=v_t,
        scalar=coef[:],
        in1=s_t,
        op0=mult,
        op1=add,
    ).then_inc(sv, 1)

    # ---- stores ----
    sp.wait_ge(sv, 36)
    sp.dma_start(out=out[0:28, :], in_=o_t[0:28, :]).then_inc(so, 16)
    act.wait_ge(sv, 36)
    act.dma_start(out=out[28:40, :], in_=o_t[28:40, :]).then_inc(so, 16)
    pl.wait_ge(sv, 36)
    pl.dma_start(out=out[40:64, :], in_=o_t[40:64, :]).then_inc(so, 16)

    # so=48: all stores landed => ACT/pool passed their sv-waits too.
    sp.wait_ge(so, 48)
    sp.sem_clear(sv)
    sp.sem_clear(ss)
    sp.sem_clear(so)

    return nc


def kernel(i=None, v=None, s=None, **_):
    global _nc
    from concourse.bass_utils import run_bass_kernel_spmd

    if _nc is None:
        _nc = _build()

    v = np.asarray(v, dtype=np.float32)
    s = np.asarray(s, dtype=np.float32)
    in_maps = [
        {
            "vs": np.ascontiguousarray(
                np.stack(
                    [v[c * B_LOC : (c + 1) * B_LOC], s[c * B_LOC : (c + 1) * B_LOC]]
                )
            )
        }
        for c in range(N_CORES)
    ]
    res = run_bass_kernel_spmd(_nc, in_maps, core_ids=list(range(N_CORES)))
    return np.concatenate([r["out"] for r in res.results], axis=0)
